# revision 9
# baseline (speedup 1.0000x reference)
"""Trainium2 Bass kernel for nn_AttentionBlock (GroupNorm + 4-head self-attention
over a [4, 256, 64, 64] image batch).

Sharding: 8 cores = (batch n in 0..3) x (query-half j in 0..1). Each core gets
batch n's full [256, 4096] feature map, spatially rolled by j*2048 so that the
core's 2048 query positions are always columns 0..2047 (GroupNorm stats and
softmax sums are invariant to a consistent spatial permutation). Each core
returns its [256, 2048] output slice; the host reassembles.

Numerics: GroupNorm in fp32; all large matmuls in fp16 (1 cycle/row on the PE,
fp32 PSUM accumulation). Softmax runs without the max-subtraction pass (scores
are ~N(0,1); |s| < 15 by a huge margin, exp stays in fp16/fp32 range), with the
denominator produced by an extra all-ones column in the V operand so the AV
matmul accumulates it for free.
"""

import os
import numpy as np

C = 256
L = 4096
LQ = 2048
HEADS = 4
DH = 64
GROUPS = 32
EPS = 1e-5
N_CORES = 8

# dtype for the big matmuls: f16 (default) or bf16
MM_DT = os.environ.get("ATTN_MM_DT", "f16")


def _np_mm_dt():
    import ml_dtypes
    return np.float16 if MM_DT == "f16" else ml_dtypes.bfloat16


def _build_nc(debug=False):
    import concourse.bacc as bacc
    import concourse.tile as tile
    from concourse import mybir

    f32 = mybir.dt.float32
    f16 = mybir.dt.float16 if MM_DT == "f16" else mybir.dt.bfloat16

    AF = mybir.ActivationFunctionType
    OP = mybir.AluOpType

    nc = bacc.Bacc("TRN2", target_bir_lowering=False, debug=False,
                   num_devices=N_CORES)

    x_d = nc.dram_tensor("x", [C, L], f32, kind="ExternalInput").ap()
    wq_d = nc.dram_tensor("wqT", [C, C], f16, kind="ExternalInput").ap()
    wk_d = nc.dram_tensor("wkT", [C, C], f16, kind="ExternalInput").ap()
    wv_d = nc.dram_tensor("wvT", [C, C], f16, kind="ExternalInput").ap()
    wp_d = nc.dram_tensor("wpT", [C, C], f16, kind="ExternalInput").ap()
    gamma_d = nc.dram_tensor("gamma", [C], f32, kind="ExternalInput").ap()
    beta_d = nc.dram_tensor("beta", [C], f32, kind="ExternalInput").ap()
    bp_d = nc.dram_tensor("bp", [C], f32, kind="ExternalInput").ap()
    mgrp_d = nc.dram_tensor("mgrp", [128, 16], f32, kind="ExternalInput").ap()
    mbc_d = nc.dram_tensor("mbc", [16, 128], f32, kind="ExternalInput").ap()
    y_d = nc.dram_tensor("y", [C, LQ], f32, kind="ExternalOutput").ap()
    dbg = {}
    if debug:
        dbg["xn"] = nc.dram_tensor("dbg_xn", [C, L], f16, kind="ExternalOutput").ap()
        dbg["q"] = nc.dram_tensor("dbg_q", [C, LQ], f16, kind="ExternalOutput").ap()
        dbg["k"] = nc.dram_tensor("dbg_k", [C, L], f16, kind="ExternalOutput").ap()
        dbg["v"] = nc.dram_tensor("dbg_v", [128, 32 * HEADS * 65], f16,
                                  kind="ExternalOutput").ap()
        dbg["on"] = nc.dram_tensor("dbg_on", [64, HEADS * LQ], f16,
                                   kind="ExternalOutput").ap()

    with tile.TileContext(nc) as tc:
        with (
            tc.tile_pool(name="consts", bufs=1) as cpool,
            tc.tile_pool(name="xp", bufs=1) as xpool,
            tc.tile_pool(name="xnp", bufs=1) as xnpool,
            tc.tile_pool(name="kp", bufs=1) as kpool,
            tc.tile_pool(name="qp", bufs=1) as qpool,
            tc.tile_pool(name="vp", bufs=1) as vpool,
            tc.tile_pool(name="small", bufs=2) as spool,
            tc.tile_pool(name="ptp", bufs=3) as ptpool,
            tc.tile_pool(name="onp", bufs=6) as opool,
            tc.tile_pool(name="yp", bufs=2) as ypool,
            tc.tile_pool(name="psA", bufs=1, space="PSUM") as psA,
            tc.tile_pool(name="psB", bufs=2, space="PSUM") as psB,
        ):
            # PSUM: one manually-banked ring tile (6 banks); Tile tracks
            # dependencies at bank granularity within it. The remaining 2
            # banks hold the two AV accumulators of the active head pair.
            ring = psA.tile([128, 6, 512], f32, tag="ring")
            ring_ctr = [0]

            def rslot():
                r = ring_ctr[0] % 6
                ring_ctr[0] += 1
                return ring[:, r, :]

            # ---------------- input DMAs ----------------
            xt = []
            for t in range(2):
                xx = xpool.tile([128, L], f32, tag=f"x{t}", name=f"x{t}")
                nc.sync.dma_start(xx[:], x_d[t * 128:(t + 1) * 128, :])
                xt.append(xx)

            wq_sb = cpool.tile([128, 2, C], f16, tag="wq")
            nc.sync.dma_start(wq_sb[:], wq_d.rearrange("(k p) o -> p k o", k=2))
            wk_sb = cpool.tile([128, 2, C], f16, tag="wk")
            nc.sync.dma_start(wk_sb[:], wk_d.rearrange("(k p) o -> p k o", k=2))
            wv_sb = cpool.tile([128, 2, C], f16, tag="wv")
            nc.sync.dma_start(wv_sb[:], wv_d.rearrange("(k p) o -> p k o", k=2))
            wp_sb = cpool.tile([64, HEADS, C], f16, tag="wp")
            nc.sync.dma_start(wp_sb[:], wp_d.rearrange("(h p) o -> p h o", h=4))

            gcol = cpool.tile([128, 2], f32, tag="gcol")
            nc.sync.dma_start(gcol[:], gamma_d.rearrange("(t p) -> p t", t=2))
            bcol = cpool.tile([128, 2], f32, tag="bcol")
            nc.sync.dma_start(bcol[:], beta_d.rearrange("(t p) -> p t", t=2))
            bpcol = cpool.tile([128, 2], f32, tag="bpcol")
            nc.sync.dma_start(bpcol[:], bp_d.rearrange("(t p) -> p t", t=2))
            mgrp_sb = cpool.tile([128, 16], f32, tag="mgrp")
            nc.sync.dma_start(mgrp_sb[:], mgrp_d[:])
            mbc_sb = cpool.tile([16, 128], f32, tag="mbc")
            nc.sync.dma_start(mbc_sb[:], mbc_d[:])

            ones_sb = cpool.tile([1, 64], f16, tag="ones")
            nc.vector.memset(ones_sb[:], 1.0)

            # ---------------- GroupNorm (fp32) ----------------
            # Per-channel mean/E[x^2] via bn_stats, group-aggregated via a tiny
            # PE matmul with the (1/8) group-membership matrix, broadcast back
            # with its transpose, applied as a per-partition affine.
            sb4 = spool.tile([128, 4], f32, tag="sb4")
            for t in range(2):
                stats = spool.tile([128, 8, 6], f32, tag="stats")
                xv = xt[t].rearrange("p (s f) -> p s f", f=512)
                for s in range(8):
                    nc.vector.bn_stats(stats[:, s, :], xv[:, s, :])
                mv = spool.tile([128, 2], f32, tag="mv")
                nc.vector.bn_aggr(mv[:], stats[:])
                nc.vector.tensor_copy(sb4[:, 2 * t:2 * t + 1], mv[:, 0:1])
                tmp = spool.tile([128, 1], f32, tag="tmp1")
                nc.vector.tensor_mul(tmp[:], mv[:, 0:1], mv[:, 0:1])
                nc.vector.tensor_add(sb4[:, 2 * t + 1:2 * t + 2], mv[:, 1:2], tmp[:])

            gps = ring[0:16, 0, 0:4]
            nc.tensor.matmul(gps, mgrp_sb[:], sb4[:], start=True, stop=True)
            gsb = spool.tile([16, 4], f32, tag="gsb")
            nc.vector.tensor_copy(gsb[:], gps)

            # gb2 cols: mean_t0, rstd_t0, mean_t1, rstd_t1
            gb2 = spool.tile([16, 4], f32, tag="gb2")
            gw = spool.tile([16, 8], f32, tag="gw")
            for t in range(2):
                m = gsb[:, 2 * t:2 * t + 1]
                e = gsb[:, 2 * t + 1:2 * t + 2]
                msq = gw[:, t:t + 1]
                nc.vector.tensor_mul(msq, m, m)
                w = gw[:, 2 + t:3 + t]
                nc.vector.tensor_sub(w, e, msq)                  # var
                nc.vector.tensor_scalar_add(w, w, EPS)           # var + eps
                s_ = gw[:, 4 + t:5 + t]
                nc.scalar.activation(s_, w, AF.Sqrt)             # sqrt(var+eps)
                r_ = gb2[:, 2 * t + 1:2 * t + 2]
                nc.vector.reciprocal(r_, s_)                     # ~rsqrt
                # one Newton step: r *= 1.5 - 0.5 * w * r^2
                t2 = gw[:, 6 + t:7 + t]
                nc.vector.tensor_mul(t2, r_, r_)
                nc.vector.tensor_mul(t2, t2, w)
                nc.vector.tensor_scalar(t2, t2, -0.5, 1.5, op0=OP.mult, op1=OP.add)
                nc.vector.tensor_mul(r_, r_, t2)
                nc.vector.tensor_copy(gb2[:, 2 * t:2 * t + 1], m)

            bps = ring[:, 1, 0:4]
            nc.tensor.matmul(bps, mbc_sb[:], gb2[:], start=True, stop=True)
            pcol = spool.tile([128, 4], f32, tag="pcol")
            nc.vector.tensor_copy(pcol[:], bps)

            xn = []
            for t in range(2):
                nc.vector.tensor_scalar(
                    xt[t][:], xt[t][:],
                    pcol[:, 2 * t:2 * t + 1], pcol[:, 2 * t + 1:2 * t + 2],
                    op0=OP.subtract, op1=OP.mult)
                x16 = xnpool.tile([128, L], f16, tag=f"xn{t}", name=f"xn{t}")
                nc.vector.tensor_scalar(
                    x16[:], xt[t][:],
                    gcol[:, t:t + 1], bcol[:, t:t + 1],
                    op0=OP.mult, op1=OP.add)
                xn.append(x16)
            if debug:
                for t in range(2):
                    nc.sync.dma_start(dbg["xn"][t * 128:(t + 1) * 128, :], xn[t][:])

            # ---------------- Q/K projections (channel-major, fp16) ------------
            qt = [qpool.tile([128, LQ], f16, tag=f"q{t}", name=f"q{t}")
                  for t in range(2)]
            kt = [kpool.tile([128, L], f16, tag=f"k{t}", name=f"k{t}")
                  for t in range(2)]
            for (dst, w_sb, ncols) in ((qt, wq_sb, LQ), (kt, wk_sb, L)):
                for t in range(2):
                    for c0 in range(0, ncols, 512):
                        ps = rslot()
                        for kk in range(2):
                            nc.tensor.matmul(
                                ps,
                                w_sb[:, kk, t * 128:(t + 1) * 128],
                                xn[kk][:, c0:c0 + 512],
                                start=(kk == 0), stop=(kk == 1))
                        nc.vector.tensor_copy(dst[t][:, c0:c0 + 512], ps)
            if debug:
                for t in range(2):
                    nc.sync.dma_start(dbg["q"][t * 128:(t + 1) * 128, :], qt[t][:])
                    nc.sync.dma_start(dbg["k"][t * 128:(t + 1) * 128, :], kt[t][:])

            # ---------------- V projection, directly row-major ----------------
            # v_aug[p, j, h, 0:64] = v_h[m = j*128+p, d]; col 64 = 1.0 (softmax
            # denominator row produced for free by the AV matmul).
            v_aug = vpool.tile([128, 32, HEADS, 65], f16, tag="vaug")
            nc.vector.memset(v_aug[:, :, :, 64:65], 1.0)
            for j in range(32):
                ps = rslot()[:, 0:256]
                for kk in range(2):
                    nc.tensor.matmul(
                        ps,
                        xn[kk][:, j * 128:(j + 1) * 128],
                        wv_sb[:, kk, :],
                        start=(kk == 0), stop=(kk == 1))
                nc.vector.tensor_copy(
                    v_aug[:, j, :, 0:64],
                    ps.rearrange("p (h d) -> p h d", h=HEADS))
            if debug:
                nc.sync.dma_start(
                    dbg["v"][:], v_aug.rearrange("p a b c -> p (a b c)"))

            # ---------------- attention ----------------
            # Heads run in row-tiled pairs: head 2t occupies PE rows 0-63 and
            # head 2t+1 rows 64-127 (their channel-major partition homes), so
            # the two K=64 score matmuls run CONCURRENTLY in the array.
            # Score slices cycle through the 6-bank PSUM ring; exp fires on
            # aligned groups of 3 banks (1536 cols per ACTIVATE). The two AV
            # accumulators use the remaining 2 banks.
            for lc in range(4):            # lq chunks of 512
                q0 = lc * 512
                onh_tiles = {}
                for t in range(2):         # head pair (2t, 2t+1)
                    av = [psB.tile([65, 512], f32, tag="av", name=f"av{hh}")
                          for hh in range(2)]
                    # 64 slices: slice i = (j, hh); ring half b = (i//3) % 2
                    # holds 3 slices; exp + AV fire per full group.
                    group = []             # [(ring_idx, j, hh)]
                    gbase = None
                    for i in range(64):
                        j, hh = i // 2, i % 2
                        r = i % 6
                        if i % 3 == 0:
                            gbase = r
                        nc.tensor.matmul(
                            ring[:, r, :],
                            kt[t][64 * hh:64 * hh + 64, j * 128:(j + 1) * 128],
                            qt[t][64 * hh:64 * hh + 64, q0:q0 + 512],
                            start=True, stop=True)
                        group.append((r, j, hh))
                        if len(group) == 3 or i == 63:
                            ng = len(group)
                            pt = ptpool.tile([128, 3 * 512], f16, tag="pt",
                                             name="pt")
                            src_ap = ring[:, gbase:gbase + ng, :].rearrange(
                                "p a b -> p (a b)")
                            nc.scalar.activation(pt[:, :ng * 512], src_ap, AF.Exp)
                            for gi, (r, ji, hi) in enumerate(group):
                                nc.tensor.matmul(
                                    av[hi][:],
                                    v_aug[:, ji, 2 * t + hi, :],
                                    pt[:, gi * 512:(gi + 1) * 512],
                                    start=(ji == 0), stop=(ji == 31),
                                    skip_group_check=True)
                            group = []
                    # normalize: out = av[0:64] * (1 / av[64]): broadcast the
                    # raw fp16 denominator with a PE outer product, reciprocal
                    # on 64 partitions, multiply. Emit the PE bc ops first so
                    # the PE can move on to the next pair's scores quickly.
                    ring_ctr[0] = 4   # bc -> slots 4,5 (oldest); yproj -> 0,1
                    d16s, bcs_ps = [], []
                    for hh in range(2):
                        d16 = spool.tile([1, 512], f16, tag="d16", name="d16")
                        nc.vector.tensor_copy(d16[:], av[hh][64:65, :])
                        d16s.append(d16)
                    for hh in range(2):
                        bc = rslot()[0:64, :]
                        nc.tensor.matmul(bc, ones_sb[:], d16s[hh][:],
                                         start=True, stop=True)
                        bcs_ps.append(bc)
                    for hh in range(2):
                        h = 2 * t + hh
                        bc_sb = spool.tile([64, 512], f32, tag="bcsb",
                                           name="bcsb")
                        nc.vector.tensor_copy(bc_sb[:], bcs_ps[hh])
                        rb = spool.tile([64, 512], f32, tag="rb", name="rb")
                        nc.vector.reciprocal_approx_fast(rb[:], bc_sb[:])
                        onh = opool.tile([64, 512], f16, tag="onh", name="onh")
                        nc.vector.tensor_mul(onh[:], av[hh][0:64, :], rb[:])
                        onh_tiles[h] = onh
                        if debug:
                            nc.sync.dma_start(
                                dbg["on"][:, h * LQ + q0:h * LQ + q0 + 512],
                                onh[:])

                # output projection for this lq chunk, accumulated over heads
                for o in range(2):
                    yps = rslot()
                    for h in range(HEADS):
                        nc.tensor.matmul(
                            yps,
                            wp_sb[:, h, o * 128:(o + 1) * 128],
                            onh_tiles[h][:],
                            start=(h == 0), stop=(h == HEADS - 1),
                            skip_group_check=True)
                    ysb = ypool.tile([128, 512], f32, tag="ysb")
                    nc.vector.tensor_scalar_add(ysb[:], yps, bpcol[:, o:o + 1])
                    nc.sync.dma_start(
                        y_d[o * 128:(o + 1) * 128, q0:q0 + 512], ysb[:])

    nc.compile()
    return nc


_NC_CACHE = {}


def _get_nc(debug=False):
    key = (debug, MM_DT)
    if key not in _NC_CACHE:
        _NC_CACHE[key] = _build_nc(debug=debug)
    return _NC_CACHE[key]


def _host_inputs(x, gamma, beta, Wq, Wk, Wv, Wp, bp):
    """Build the 8 per-core input maps."""
    x = np.asarray(x, np.float32).reshape(4, C, L)
    scale = DH ** -0.5
    mmdt = _np_mm_dt()
    shared = {
        "wqT": np.ascontiguousarray((np.asarray(Wq, np.float32) * scale).T).astype(mmdt),
        "wkT": np.ascontiguousarray(np.asarray(Wk, np.float32).T).astype(mmdt),
        "wvT": np.ascontiguousarray(np.asarray(Wv, np.float32).T).astype(mmdt),
        "wpT": np.ascontiguousarray(np.asarray(Wp, np.float32).T).astype(mmdt),
        "gamma": np.asarray(gamma, np.float32),
        "beta": np.asarray(beta, np.float32),
        "bp": np.asarray(bp, np.float32),
        "mgrp": _mgrp(),
        "mbc": _mbc(),
    }
    in_maps = []
    for c in range(N_CORES):
        n, j = c // 2, c % 2
        xf = x[n]
        xr = np.ascontiguousarray(
            np.concatenate([xf[:, j * LQ:], xf[:, :j * LQ]], axis=1))
        in_maps.append({"x": xr, **shared})
    return in_maps


def _mgrp():
    m = np.zeros((128, 16), np.float32)
    for p in range(128):
        m[p, p // 8] = 1.0 / 8.0
    return m


def _mbc():
    m = np.zeros((16, 128), np.float32)
    for p in range(128):
        m[p // 8, p] = 1.0
    return m


def _assemble(results):
    y = np.zeros((4, C, L), np.float32)
    for c in range(N_CORES):
        n, j = c // 2, c % 2
        y[n][:, j * LQ:(j + 1) * LQ] = results[c]["y"]
    return y.reshape(4, C, 64, 64)


def kernel(x, gamma, beta, Wq, Wk, Wv, Wp, bp):
    from concourse.bass_utils import run_bass_kernel_spmd

    nc = _get_nc()
    in_maps = _host_inputs(x, gamma, beta, Wq, Wk, Wv, Wp, bp)
    res = run_bass_kernel_spmd(nc, in_maps, core_ids=list(range(N_CORES)))
    return _assemble(res.results)


# revision 12
# speedup vs baseline: 2.3042x; 2.3042x over previous
"""Trainium2 Bass kernel for nn_AttentionBlock (GroupNorm + 4-head self-attention
over a [4, 256, 64, 64] image batch).

Sharding: 8 cores = (batch n in 0..3) x (query-half j in 0..1). Each core gets
batch n's full [256, 4096] feature map, spatially rolled by j*2048 so that the
core's 2048 query positions are always columns 0..2047 (GroupNorm stats and
softmax sums are invariant to a consistent spatial permutation). Each core
returns its [256, 2048] output slice; the host reassembles.

Numerics: GroupNorm in fp32; all large matmuls in fp16 (1 cycle/row on the PE,
fp32 PSUM accumulation). Softmax runs without the max-subtraction pass (scores
are ~N(0,1); |s| < 15 by a huge margin, exp stays in fp16/fp32 range), with the
denominator produced by an extra all-ones column in the V operand so the AV
matmul accumulates it for free.
"""

import os
import numpy as np

C = 256
L = 4096
LQ = 2048
HEADS = 4
DH = 64
GROUPS = 32
EPS = 1e-5
N_CORES = 8

# dtype for the big matmuls: f16 (default) or bf16
MM_DT = os.environ.get("ATTN_MM_DT", "f16")


def _np_mm_dt():
    import ml_dtypes
    return np.float16 if MM_DT == "f16" else ml_dtypes.bfloat16


def _build_nc(debug=False):
    import concourse.bacc as bacc
    import concourse.tile as tile
    from concourse import mybir

    f32 = mybir.dt.float32
    f16 = mybir.dt.float16 if MM_DT == "f16" else mybir.dt.bfloat16

    AF = mybir.ActivationFunctionType
    OP = mybir.AluOpType

    nc = bacc.Bacc("TRN2", target_bir_lowering=False, debug=False,
                   num_devices=N_CORES)

    x_d = nc.dram_tensor("x", [C, L], f32, kind="ExternalInput").ap()
    wq_d = nc.dram_tensor("wqT", [C, C], f16, kind="ExternalInput").ap()
    wk_d = nc.dram_tensor("wkT", [C, C], f16, kind="ExternalInput").ap()
    wv_d = nc.dram_tensor("wvT", [C, C], f16, kind="ExternalInput").ap()
    wp_d = nc.dram_tensor("wpT", [C, C], f16, kind="ExternalInput").ap()
    gamma_d = nc.dram_tensor("gamma", [C], f32, kind="ExternalInput").ap()
    beta_d = nc.dram_tensor("beta", [C], f32, kind="ExternalInput").ap()
    bp_d = nc.dram_tensor("bp", [C], f32, kind="ExternalInput").ap()
    mgrp_d = nc.dram_tensor("mgrp", [128, 16], f32, kind="ExternalInput").ap()
    mbc_d = nc.dram_tensor("mbc", [16, 128], f32, kind="ExternalInput").ap()
    y_d = nc.dram_tensor("y", [C, LQ], f32, kind="ExternalOutput").ap()
    dbg = {}
    if debug:
        dbg["xn"] = nc.dram_tensor("dbg_xn", [C, L], f16, kind="ExternalOutput").ap()
        dbg["q"] = nc.dram_tensor("dbg_q", [C, LQ], f16, kind="ExternalOutput").ap()
        dbg["k"] = nc.dram_tensor("dbg_k", [C, L], f16, kind="ExternalOutput").ap()
        dbg["v"] = nc.dram_tensor("dbg_v", [128, 32 * HEADS * 65], f16,
                                  kind="ExternalOutput").ap()
        dbg["on"] = nc.dram_tensor("dbg_on", [64, HEADS * LQ], f16,
                                   kind="ExternalOutput").ap()

    with tile.TileContext(nc) as tc:
        with (
            tc.tile_pool(name="consts", bufs=1) as cpool,
            tc.tile_pool(name="xp", bufs=1) as xpool,
            tc.tile_pool(name="xnp", bufs=1) as xnpool,
            tc.tile_pool(name="kp", bufs=1) as kpool,
            tc.tile_pool(name="qp", bufs=1) as qpool,
            tc.tile_pool(name="vp", bufs=1) as vpool,
            tc.tile_pool(name="small", bufs=2) as spool,
            tc.tile_pool(name="ptp", bufs=3) as ptpool,
            tc.tile_pool(name="onp", bufs=10) as opool,
            tc.tile_pool(name="yp", bufs=2) as ypool,
            tc.tile_pool(name="psA", bufs=2, space="PSUM") as psA,
            tc.tile_pool(name="psB", bufs=2, space="PSUM") as psB,
        ):
            # ---------------- input DMAs ----------------
            xt = []
            for t in range(2):
                xx = xpool.tile([128, L], f32, tag=f"x{t}", name=f"x{t}")
                nc.sync.dma_start(xx[:], x_d[t * 128:(t + 1) * 128, :])
                xt.append(xx)

            wq_sb = cpool.tile([128, 2, C], f16, tag="wq")
            nc.sync.dma_start(wq_sb[:], wq_d.rearrange("(k p) o -> p k o", k=2))
            wk_sb = cpool.tile([128, 2, C], f16, tag="wk")
            nc.sync.dma_start(wk_sb[:], wk_d.rearrange("(k p) o -> p k o", k=2))
            wv_sb = cpool.tile([128, 2, C], f16, tag="wv")
            nc.sync.dma_start(wv_sb[:], wv_d.rearrange("(k p) o -> p k o", k=2))
            wp_sb = cpool.tile([64, HEADS, C], f16, tag="wp")
            nc.sync.dma_start(wp_sb[:], wp_d.rearrange("(h p) o -> p h o", h=4))

            gcol = cpool.tile([128, 2], f32, tag="gcol")
            nc.sync.dma_start(gcol[:], gamma_d.rearrange("(t p) -> p t", t=2))
            bcol = cpool.tile([128, 2], f32, tag="bcol")
            nc.sync.dma_start(bcol[:], beta_d.rearrange("(t p) -> p t", t=2))
            bpcol = cpool.tile([128, 2], f32, tag="bpcol")
            nc.sync.dma_start(bpcol[:], bp_d.rearrange("(t p) -> p t", t=2))
            mgrp_sb = cpool.tile([128, 16], f32, tag="mgrp")
            nc.sync.dma_start(mgrp_sb[:], mgrp_d[:])
            mbc_sb = cpool.tile([16, 128], f32, tag="mbc")
            nc.sync.dma_start(mbc_sb[:], mbc_d[:])

            ones_sb = cpool.tile([1, 64], f16, tag="ones")
            nc.vector.memset(ones_sb[:], 1.0)

            # ---------------- GroupNorm (fp32) ----------------
            # Per-channel mean/E[x^2] via bn_stats, group-aggregated via a tiny
            # PE matmul with the (1/8) group-membership matrix, broadcast back
            # with its transpose, applied as a per-partition affine.
            sb4 = spool.tile([128, 4], f32, tag="sb4")
            for t in range(2):
                stats = spool.tile([128, 8, 6], f32, tag="stats")
                xv = xt[t].rearrange("p (s f) -> p s f", f=512)
                for s in range(8):
                    nc.vector.bn_stats(stats[:, s, :], xv[:, s, :])
                mv = spool.tile([128, 2], f32, tag="mv")
                nc.vector.bn_aggr(mv[:], stats[:])
                nc.vector.tensor_copy(sb4[:, 2 * t:2 * t + 1], mv[:, 0:1])
                tmp = spool.tile([128, 1], f32, tag="tmp1")
                nc.vector.tensor_mul(tmp[:], mv[:, 0:1], mv[:, 0:1])
                nc.vector.tensor_add(sb4[:, 2 * t + 1:2 * t + 2], mv[:, 1:2], tmp[:])

            gps = psA.tile([16, 4], f32, tag="sc")
            nc.tensor.matmul(gps[:], mgrp_sb[:], sb4[:], start=True, stop=True)
            gsb = spool.tile([16, 4], f32, tag="gsb")
            nc.vector.tensor_copy(gsb[:], gps[:])

            # gb2 cols: mean_t0, rstd_t0, mean_t1, rstd_t1
            gb2 = spool.tile([16, 4], f32, tag="gb2")
            gw = spool.tile([16, 8], f32, tag="gw")
            for t in range(2):
                m = gsb[:, 2 * t:2 * t + 1]
                e = gsb[:, 2 * t + 1:2 * t + 2]
                msq = gw[:, t:t + 1]
                nc.vector.tensor_mul(msq, m, m)
                w = gw[:, 2 + t:3 + t]
                nc.vector.tensor_sub(w, e, msq)                  # var
                nc.vector.tensor_scalar_add(w, w, EPS)           # var + eps
                s_ = gw[:, 4 + t:5 + t]
                nc.scalar.activation(s_, w, AF.Sqrt)             # sqrt(var+eps)
                r_ = gb2[:, 2 * t + 1:2 * t + 2]
                nc.vector.reciprocal(r_, s_)                     # ~rsqrt
                # one Newton step: r *= 1.5 - 0.5 * w * r^2
                t2 = gw[:, 6 + t:7 + t]
                nc.vector.tensor_mul(t2, r_, r_)
                nc.vector.tensor_mul(t2, t2, w)
                nc.vector.tensor_scalar(t2, t2, -0.5, 1.5, op0=OP.mult, op1=OP.add)
                nc.vector.tensor_mul(r_, r_, t2)
                nc.vector.tensor_copy(gb2[:, 2 * t:2 * t + 1], m)

            bps = psA.tile([128, 4], f32, tag="sc")
            nc.tensor.matmul(bps[:], mbc_sb[:], gb2[:], start=True, stop=True)
            pcol = spool.tile([128, 4], f32, tag="pcol")
            nc.vector.tensor_copy(pcol[:], bps[:])

            xn = []
            for t in range(2):
                nc.vector.tensor_scalar(
                    xt[t][:], xt[t][:],
                    pcol[:, 2 * t:2 * t + 1], pcol[:, 2 * t + 1:2 * t + 2],
                    op0=OP.subtract, op1=OP.mult)
                x16 = xnpool.tile([128, L], f16, tag=f"xn{t}", name=f"xn{t}")
                nc.vector.tensor_scalar(
                    x16[:], xt[t][:],
                    gcol[:, t:t + 1], bcol[:, t:t + 1],
                    op0=OP.mult, op1=OP.add)
                xn.append(x16)
            if debug:
                for t in range(2):
                    nc.sync.dma_start(dbg["xn"][t * 128:(t + 1) * 128, :], xn[t][:])

            # ---------------- Q/K projections (channel-major, fp16) ------------
            qt = [qpool.tile([128, LQ], f16, tag=f"q{t}", name=f"q{t}")
                  for t in range(2)]
            kt = [kpool.tile([128, L], f16, tag=f"k{t}", name=f"k{t}")
                  for t in range(2)]
            for (dst, w_sb, ncols) in ((qt, wq_sb, LQ), (kt, wk_sb, L)):
                for t in range(2):
                    for c0 in range(0, ncols, 512):
                        ps = psA.tile([128, 512], f32, tag="sc")
                        for kk in range(2):
                            nc.tensor.matmul(
                                ps[:],
                                w_sb[:, kk, t * 128:(t + 1) * 128],
                                xn[kk][:, c0:c0 + 512],
                                start=(kk == 0), stop=(kk == 1))
                        nc.vector.tensor_copy(dst[t][:, c0:c0 + 512], ps[:])
            if debug:
                for t in range(2):
                    nc.sync.dma_start(dbg["q"][t * 128:(t + 1) * 128, :], qt[t][:])
                    nc.sync.dma_start(dbg["k"][t * 128:(t + 1) * 128, :], kt[t][:])

            # ---------------- V projection, directly row-major ----------------
            # v_aug[p, j, h, 0:64] = v_h[m = j*128+p, d]; col 64 = 1.0 (softmax
            # denominator row produced for free by the AV matmul).
            v_aug = vpool.tile([128, 32, HEADS, 65], f16, tag="vaug")
            nc.vector.memset(v_aug[:, :, :, 64:65], 1.0)
            for j in range(32):
                ps = psB.tile([128, 256], f32, tag="av")
                for kk in range(2):
                    nc.tensor.matmul(
                        ps[:],
                        xn[kk][:, j * 128:(j + 1) * 128],
                        wv_sb[:, kk, :],
                        start=(kk == 0), stop=(kk == 1))
                nc.vector.tensor_copy(
                    v_aug[:, j, :, 0:64],
                    ps.rearrange("p (h d) -> p h d", h=HEADS))
            if debug:
                nc.sync.dma_start(
                    dbg["v"][:], v_aug.rearrange("p a b c -> p (a b c)"))

            # ---------------- attention ----------------
            # Heads run in row-tiled pairs: head 2t occupies PE rows 0-63 and
            # head 2t+1 rows 64-127 (their channel-major partition homes), so
            # the two K=64 score matmuls run CONCURRENTLY in the array.
            # Score slices fill [128, 1536] PSUM tiles (3 banks, double
            # buffered); one exp ACTIVATE covers 3 slices. The softmax
            # denominator row is broadcast with a partition-broadcast DMA (no
            # PE involvement), so the PE rolls straight into the next pair.
            pending_proj = []

            def emit_proj():
                if not pending_proj:
                    return
                q0p, tiles = pending_proj.pop()
                for o in range(2):
                    yps = psB.tile([128, 512], f32, tag="av", name="yps")
                    for h in range(HEADS):
                        nc.tensor.matmul(
                            yps[:],
                            wp_sb[:, h, o * 128:(o + 1) * 128],
                            tiles[h][:],
                            start=(h == 0), stop=(h == HEADS - 1),
                            skip_group_check=True)
                    ysb = ypool.tile([128, 512], f32, tag="ysb", name="ysb")
                    nc.vector.tensor_scalar_add(ysb[:], yps[:], bpcol[:, o:o + 1])
                    nc.sync.dma_start(
                        y_d[o * 128:(o + 1) * 128, q0p:q0p + 512], ysb[:])

            for lc in range(4):            # lq chunks of 512
                q0 = lc * 512
                onh_tiles = {}
                for t in range(2):         # head pair (2t, 2t+1)
                    if t == 1:
                        # pipeline: previous lq-chunk's output projection runs
                        # between the two pairs, reusing the freed av slots and
                        # bridging the PE gap across the pair-0 normalize.
                        emit_proj()
                    av = [psB.tile([65, 512], f32, tag="av", name=f"av{hh}")
                          for hh in range(2)]
                    group = []             # [(slice_in_tile, j, hh)]
                    sc = None
                    for i in range(64):
                        j, hh = i // 2, i % 2
                        if not group:
                            ns = min(3, 64 - i)
                            sc = psA.tile([128, ns * 512], f32,
                                          tag="sc", name="sc")
                        s = len(group)
                        nc.tensor.matmul(
                            sc[:, s * 512:(s + 1) * 512],
                            kt[t][64 * hh:64 * hh + 64, j * 128:(j + 1) * 128],
                            qt[t][64 * hh:64 * hh + 64, q0:q0 + 512],
                            start=True, stop=True)
                        group.append((s, j, hh))
                        if len(group) * 512 == sc.shape[1] or i == 63:
                            ng = len(group)
                            pt = ptpool.tile([128, ng * 512], f16, tag="pt",
                                             name="pt")
                            nc.scalar.activation(pt[:], sc[:], AF.Exp)
                            for (s, ji, hi) in group:
                                nc.tensor.matmul(
                                    av[hi][:],
                                    v_aug[:, ji, 2 * t + hi, :],
                                    pt[:, s * 512:(s + 1) * 512],
                                    start=(ji == 0), stop=(ji == 31),
                                    skip_group_check=True)
                            group = []
                    # normalize: out = av[0:64] * (1 / av[64]); denominator
                    # row -> DMA partition-broadcast -> reciprocal -> multiply
                    # (DVE + DMA only; PE proceeds with the next pair).
                    for hh in range(2):
                        h = 2 * t + hh
                        d32 = spool.tile([1, 512], f32, tag="d32", name="d32")
                        nc.vector.tensor_copy(d32[:], av[hh][64:65, :])
                        bc_sb = spool.tile([64, 512], f32, tag="bcsb",
                                           name="bcsb")
                        nc.gpsimd.partition_broadcast(bc_sb[:], d32[:])
                        rb = spool.tile([64, 512], f32, tag="rb", name="rb")
                        nc.vector.reciprocal_approx_fast(rb[:], bc_sb[:])
                        onh = opool.tile([64, 512], f16, tag="onh", name="onh")
                        nc.vector.tensor_mul(onh[:], av[hh][0:64, :], rb[:])
                        onh_tiles[h] = onh
                        if debug:
                            nc.sync.dma_start(
                                dbg["on"][:, h * LQ + q0:h * LQ + q0 + 512],
                                onh[:])
                pending_proj.append((q0, onh_tiles))
            emit_proj()

    nc.compile()
    return nc


_NC_CACHE = {}


def _get_nc(debug=False):
    key = (debug, MM_DT)
    if key not in _NC_CACHE:
        _NC_CACHE[key] = _build_nc(debug=debug)
    return _NC_CACHE[key]


def _host_inputs(x, gamma, beta, Wq, Wk, Wv, Wp, bp):
    """Build the 8 per-core input maps."""
    x = np.asarray(x, np.float32).reshape(4, C, L)
    scale = DH ** -0.5
    mmdt = _np_mm_dt()
    shared = {
        "wqT": np.ascontiguousarray((np.asarray(Wq, np.float32) * scale).T).astype(mmdt),
        "wkT": np.ascontiguousarray(np.asarray(Wk, np.float32).T).astype(mmdt),
        "wvT": np.ascontiguousarray(np.asarray(Wv, np.float32).T).astype(mmdt),
        "wpT": np.ascontiguousarray(np.asarray(Wp, np.float32).T).astype(mmdt),
        "gamma": np.asarray(gamma, np.float32),
        "beta": np.asarray(beta, np.float32),
        "bp": np.asarray(bp, np.float32),
        "mgrp": _mgrp(),
        "mbc": _mbc(),
    }
    in_maps = []
    for c in range(N_CORES):
        n, j = c // 2, c % 2
        xf = x[n]
        xr = np.ascontiguousarray(
            np.concatenate([xf[:, j * LQ:], xf[:, :j * LQ]], axis=1))
        in_maps.append({"x": xr, **shared})
    return in_maps


def _mgrp():
    m = np.zeros((128, 16), np.float32)
    for p in range(128):
        m[p, p // 8] = 1.0 / 8.0
    return m


def _mbc():
    m = np.zeros((16, 128), np.float32)
    for p in range(128):
        m[p // 8, p] = 1.0
    return m


def _assemble(results):
    y = np.zeros((4, C, L), np.float32)
    for c in range(N_CORES):
        n, j = c // 2, c % 2
        y[n][:, j * LQ:(j + 1) * LQ] = results[c]["y"]
    return y.reshape(4, C, 64, 64)


def kernel(x, gamma, beta, Wq, Wk, Wv, Wp, bp):
    from concourse.bass_utils import run_bass_kernel_spmd

    nc = _get_nc()
    in_maps = _host_inputs(x, gamma, beta, Wq, Wk, Wv, Wp, bp)
    res = run_bass_kernel_spmd(nc, in_maps, core_ids=list(range(N_CORES)))
    return _assemble(res.results)


# revision 13
# speedup vs baseline: 2.4194x; 1.0500x over previous
"""Trainium2 Bass kernel for nn_AttentionBlock (GroupNorm + 4-head self-attention
over a [4, 256, 64, 64] image batch).

Sharding: 8 cores = (batch n in 0..3) x (query-half j in 0..1). Each core gets
batch n's full [256, 4096] feature map, spatially rolled by j*2048 so that the
core's 2048 query positions are always columns 0..2047 (GroupNorm stats and
softmax sums are invariant to a consistent spatial permutation). Each core
returns its [256, 2048] output slice; the host reassembles.

Numerics: GroupNorm in fp32; all large matmuls in fp16 (1 cycle/row on the PE,
fp32 PSUM accumulation). Softmax runs without the max-subtraction pass (scores
are ~N(0,1); |s| < 15 by a huge margin, exp stays in fp16/fp32 range), with the
denominator produced by an extra all-ones column in the V operand so the AV
matmul accumulates it for free.
"""

import os
import numpy as np

C = 256
L = 4096
LQ = 2048
HEADS = 4
DH = 64
GROUPS = 32
EPS = 1e-5
N_CORES = 8

# dtype for the big matmuls: f16 (default) or bf16
MM_DT = os.environ.get("ATTN_MM_DT", "f16")


def _np_mm_dt():
    import ml_dtypes
    return np.float16 if MM_DT == "f16" else ml_dtypes.bfloat16


def _build_nc(debug=False):
    import concourse.bacc as bacc
    import concourse.tile as tile
    from concourse import mybir

    f32 = mybir.dt.float32
    f16 = mybir.dt.float16 if MM_DT == "f16" else mybir.dt.bfloat16

    AF = mybir.ActivationFunctionType
    OP = mybir.AluOpType

    nc = bacc.Bacc("TRN2", target_bir_lowering=False, debug=False,
                   num_devices=N_CORES)

    x_d = nc.dram_tensor("x", [C, L], f32, kind="ExternalInput").ap()
    wq_d = nc.dram_tensor("wqT", [C, C], f16, kind="ExternalInput").ap()
    wk_d = nc.dram_tensor("wkT", [C, C], f16, kind="ExternalInput").ap()
    wv_d = nc.dram_tensor("wvT", [C, C], f16, kind="ExternalInput").ap()
    wp_d = nc.dram_tensor("wpT", [C, C], f16, kind="ExternalInput").ap()
    gamma_d = nc.dram_tensor("gamma", [C], f32, kind="ExternalInput").ap()
    beta_d = nc.dram_tensor("beta", [C], f32, kind="ExternalInput").ap()
    bp_d = nc.dram_tensor("bp", [C], f32, kind="ExternalInput").ap()
    mgrp_d = nc.dram_tensor("mgrp", [128, 16], f32, kind="ExternalInput").ap()
    mbc_d = nc.dram_tensor("mbc", [16, 128], f32, kind="ExternalInput").ap()
    y_d = nc.dram_tensor("y", [C, LQ], f32, kind="ExternalOutput").ap()
    dbg = {}
    if debug:
        dbg["xn"] = nc.dram_tensor("dbg_xn", [C, L], f16, kind="ExternalOutput").ap()
        dbg["q"] = nc.dram_tensor("dbg_q", [C, LQ], f16, kind="ExternalOutput").ap()
        dbg["k"] = nc.dram_tensor("dbg_k", [C, L], f16, kind="ExternalOutput").ap()
        dbg["v"] = nc.dram_tensor("dbg_v", [128, 32 * HEADS * 65], f16,
                                  kind="ExternalOutput").ap()
        dbg["on"] = nc.dram_tensor("dbg_on", [64, HEADS * LQ], f16,
                                   kind="ExternalOutput").ap()

    with tile.TileContext(nc) as tc:
        with (
            tc.tile_pool(name="consts", bufs=1) as cpool,
            tc.tile_pool(name="xp", bufs=1) as xpool,
            tc.tile_pool(name="xnp", bufs=1) as xnpool,
            tc.tile_pool(name="kp", bufs=1) as kpool,
            tc.tile_pool(name="qp", bufs=1) as qpool,
            tc.tile_pool(name="vp", bufs=1) as vpool,
            tc.tile_pool(name="small", bufs=2) as spool,
            tc.tile_pool(name="ptp", bufs=4) as ptpool,
            tc.tile_pool(name="onp", bufs=10) as opool,
            tc.tile_pool(name="yp", bufs=2) as ypool,
            tc.tile_pool(name="psA", bufs=2, space="PSUM") as psA,
            tc.tile_pool(name="psB", bufs=2, space="PSUM") as psB,
        ):
            # ---------------- input DMAs ----------------
            xt = []
            for t in range(2):
                xx = xpool.tile([128, L], f32, tag=f"x{t}", name=f"x{t}")
                nc.sync.dma_start(xx[:], x_d[t * 128:(t + 1) * 128, :])
                xt.append(xx)

            wq_sb = cpool.tile([128, 2, C], f16, tag="wq")
            nc.sync.dma_start(wq_sb[:], wq_d.rearrange("(k p) o -> p k o", k=2))
            wk_sb = cpool.tile([128, 2, C], f16, tag="wk")
            nc.sync.dma_start(wk_sb[:], wk_d.rearrange("(k p) o -> p k o", k=2))
            wv_sb = cpool.tile([128, 2, C], f16, tag="wv")
            nc.sync.dma_start(wv_sb[:], wv_d.rearrange("(k p) o -> p k o", k=2))
            wp_sb = cpool.tile([64, HEADS, C], f16, tag="wp")
            nc.sync.dma_start(wp_sb[:], wp_d.rearrange("(h p) o -> p h o", h=4))

            gcol = cpool.tile([128, 2], f32, tag="gcol")
            nc.sync.dma_start(gcol[:], gamma_d.rearrange("(t p) -> p t", t=2))
            bcol = cpool.tile([128, 2], f32, tag="bcol")
            nc.sync.dma_start(bcol[:], beta_d.rearrange("(t p) -> p t", t=2))
            bpcol = cpool.tile([128, 2], f32, tag="bpcol")
            nc.sync.dma_start(bpcol[:], bp_d.rearrange("(t p) -> p t", t=2))
            mgrp_sb = cpool.tile([128, 16], f32, tag="mgrp")
            nc.sync.dma_start(mgrp_sb[:], mgrp_d[:])
            mbc_sb = cpool.tile([16, 128], f32, tag="mbc")
            nc.sync.dma_start(mbc_sb[:], mbc_d[:])

            ones_sb = cpool.tile([1, 64], f16, tag="ones")
            nc.vector.memset(ones_sb[:], 1.0)

            # ---------------- GroupNorm (fp32) ----------------
            # Per-channel mean/E[x^2] via bn_stats, group-aggregated via a tiny
            # PE matmul with the (1/8) group-membership matrix, broadcast back
            # with its transpose, applied as a per-partition affine.
            sb4 = spool.tile([128, 4], f32, tag="sb4")
            for t in range(2):
                stats = spool.tile([128, 8, 6], f32, tag="stats")
                xv = xt[t].rearrange("p (s f) -> p s f", f=512)
                for s in range(8):
                    nc.vector.bn_stats(stats[:, s, :], xv[:, s, :])
                mv = spool.tile([128, 2], f32, tag="mv")
                nc.vector.bn_aggr(mv[:], stats[:])
                nc.vector.tensor_copy(sb4[:, 2 * t:2 * t + 1], mv[:, 0:1])
                tmp = spool.tile([128, 1], f32, tag="tmp1")
                nc.vector.tensor_mul(tmp[:], mv[:, 0:1], mv[:, 0:1])
                nc.vector.tensor_add(sb4[:, 2 * t + 1:2 * t + 2], mv[:, 1:2], tmp[:])

            gps = psA.tile([16, 4], f32, tag="sc")
            nc.tensor.matmul(gps[:], mgrp_sb[:], sb4[:], start=True, stop=True)
            gsb = spool.tile([16, 4], f32, tag="gsb")
            nc.vector.tensor_copy(gsb[:], gps[:])

            # gb2 cols: mean_t0, rstd_t0, mean_t1, rstd_t1
            gb2 = spool.tile([16, 4], f32, tag="gb2")
            gw = spool.tile([16, 8], f32, tag="gw")
            for t in range(2):
                m = gsb[:, 2 * t:2 * t + 1]
                e = gsb[:, 2 * t + 1:2 * t + 2]
                msq = gw[:, t:t + 1]
                nc.vector.tensor_mul(msq, m, m)
                w = gw[:, 2 + t:3 + t]
                nc.vector.tensor_sub(w, e, msq)                  # var
                nc.vector.tensor_scalar_add(w, w, EPS)           # var + eps
                s_ = gw[:, 4 + t:5 + t]
                nc.scalar.activation(s_, w, AF.Sqrt)             # sqrt(var+eps)
                r_ = gb2[:, 2 * t + 1:2 * t + 2]
                nc.vector.reciprocal(r_, s_)                     # ~rsqrt
                # one Newton step: r *= 1.5 - 0.5 * w * r^2
                t2 = gw[:, 6 + t:7 + t]
                nc.vector.tensor_mul(t2, r_, r_)
                nc.vector.tensor_mul(t2, t2, w)
                nc.vector.tensor_scalar(t2, t2, -0.5, 1.5, op0=OP.mult, op1=OP.add)
                nc.vector.tensor_mul(r_, r_, t2)
                nc.vector.tensor_copy(gb2[:, 2 * t:2 * t + 1], m)

            bps = psA.tile([128, 4], f32, tag="sc")
            nc.tensor.matmul(bps[:], mbc_sb[:], gb2[:], start=True, stop=True)
            pcol = spool.tile([128, 4], f32, tag="pcol")
            nc.vector.tensor_copy(pcol[:], bps[:])

            xn = []
            for t in range(2):
                nc.vector.tensor_scalar(
                    xt[t][:], xt[t][:],
                    pcol[:, 2 * t:2 * t + 1], pcol[:, 2 * t + 1:2 * t + 2],
                    op0=OP.subtract, op1=OP.mult)
                x16 = xnpool.tile([128, L], f16, tag=f"xn{t}", name=f"xn{t}")
                nc.vector.tensor_scalar(
                    x16[:], xt[t][:],
                    gcol[:, t:t + 1], bcol[:, t:t + 1],
                    op0=OP.mult, op1=OP.add)
                xn.append(x16)
            if debug:
                for t in range(2):
                    nc.sync.dma_start(dbg["xn"][t * 128:(t + 1) * 128, :], xn[t][:])

            # ---------------- Q/K projections (channel-major, fp16) ------------
            qt = [qpool.tile([128, LQ], f16, tag=f"q{t}", name=f"q{t}")
                  for t in range(2)]
            kt = [kpool.tile([128, L], f16, tag=f"k{t}", name=f"k{t}")
                  for t in range(2)]
            for (dst, w_sb, ncols) in ((qt, wq_sb, LQ), (kt, wk_sb, L)):
                for t in range(2):
                    for c0 in range(0, ncols, 512):
                        ps = psA.tile([128, 512], f32, tag="sc")
                        for kk in range(2):
                            nc.tensor.matmul(
                                ps[:],
                                w_sb[:, kk, t * 128:(t + 1) * 128],
                                xn[kk][:, c0:c0 + 512],
                                start=(kk == 0), stop=(kk == 1))
                        nc.vector.tensor_copy(dst[t][:, c0:c0 + 512], ps[:])
            if debug:
                for t in range(2):
                    nc.sync.dma_start(dbg["q"][t * 128:(t + 1) * 128, :], qt[t][:])
                    nc.sync.dma_start(dbg["k"][t * 128:(t + 1) * 128, :], kt[t][:])

            # ---------------- V projection, directly row-major ----------------
            # v_aug[p, j, h, 0:64] = v_h[m = j*128+p, d]; col 64 = 1.0 (softmax
            # denominator row produced for free by the AV matmul).
            v_aug = vpool.tile([128, 32, HEADS, 65], f16, tag="vaug")
            nc.vector.memset(v_aug[:, :, :, 64:65], 1.0)
            for j in range(32):
                ps = psB.tile([128, 256], f32, tag="av")
                for kk in range(2):
                    nc.tensor.matmul(
                        ps[:],
                        xn[kk][:, j * 128:(j + 1) * 128],
                        wv_sb[:, kk, :],
                        start=(kk == 0), stop=(kk == 1))
                nc.vector.tensor_copy(
                    v_aug[:, j, :, 0:64],
                    ps.rearrange("p (h d) -> p h d", h=HEADS))
            if debug:
                nc.sync.dma_start(
                    dbg["v"][:], v_aug.rearrange("p a b c -> p (a b c)"))

            # ---------------- attention ----------------
            # Heads run in row-tiled pairs: head 2t occupies PE rows 0-63 and
            # head 2t+1 rows 64-127 (their channel-major partition homes), so
            # the two K=64 score matmuls run CONCURRENTLY in the array.
            # Score slices fill [128, 1536] PSUM tiles (3 banks, double
            # buffered); one exp ACTIVATE covers 3 slices. The softmax
            # denominator row is broadcast with a partition-broadcast DMA (no
            # PE involvement), so the PE rolls straight into the next pair.
            pending_proj = []

            def emit_proj():
                if not pending_proj:
                    return
                q0p, tiles = pending_proj.pop()
                for o in range(2):
                    yps = psB.tile([128, 512], f32, tag="av", name="yps")
                    for h in range(HEADS):
                        nc.tensor.matmul(
                            yps[:],
                            wp_sb[:, h, o * 128:(o + 1) * 128],
                            tiles[h][:],
                            start=(h == 0), stop=(h == HEADS - 1),
                            skip_group_check=True)
                    ysb = ypool.tile([128, 512], f32, tag="ysb", name="ysb")
                    nc.vector.tensor_scalar_add(ysb[:], yps[:], bpcol[:, o:o + 1])
                    nc.sync.dma_start(
                        y_d[o * 128:(o + 1) * 128, q0p:q0p + 512], ysb[:])

            for lc in range(4):            # lq chunks of 512
                q0 = lc * 512
                onh_tiles = {}
                for t in range(2):         # head pair (2t, 2t+1)
                    if t == 1:
                        # pipeline: previous lq-chunk's output projection runs
                        # between the two pairs, reusing the freed av slots and
                        # bridging the PE gap across the pair-0 normalize.
                        emit_proj()
                    av = [psB.tile([65, 512], f32, tag="av", name=f"av{hh}")
                          for hh in range(2)]
                    group = []             # [(slice_in_tile, j, hh)]
                    sc = None
                    for i in range(64):
                        j, hh = i // 2, i % 2
                        if not group:
                            ns = min(3, 64 - i)
                            sc = psA.tile([128, ns * 512], f32,
                                          tag="sc", name="sc")
                        s = len(group)
                        nc.tensor.matmul(
                            sc[:, s * 512:(s + 1) * 512],
                            kt[t][64 * hh:64 * hh + 64, j * 128:(j + 1) * 128],
                            qt[t][64 * hh:64 * hh + 64, q0:q0 + 512],
                            start=True, stop=True)
                        group.append((s, j, hh))
                        if len(group) * 512 == sc.shape[1] or i == 63:
                            ng = len(group)
                            pt = ptpool.tile([128, ng * 512], f16, tag="pt",
                                             name="pt")
                            nc.scalar.activation(pt[:], sc[:], AF.Exp)
                            for (s, ji, hi) in group:
                                nc.tensor.matmul(
                                    av[hi][:],
                                    v_aug[:, ji, 2 * t + hi, :],
                                    pt[:, s * 512:(s + 1) * 512],
                                    start=(ji == 0), stop=(ji == 31),
                                    skip_group_check=True)
                            group = []
                    # normalize: out = av[0:64] * (1 / av[64]); denominator
                    # row -> DMA partition-broadcast -> reciprocal -> multiply
                    # (DVE + DMA only; PE proceeds with the next pair).
                    # evacuate both accumulators to SBUF promptly so the
                    # PSUM slots free up for the next pair's accumulation
                    av_sbs, d32s = [], []
                    for hh in range(2):
                        d32 = spool.tile([1, 512], f32, tag="d32", name="d32")
                        nc.vector.tensor_copy(d32[:], av[hh][64:65, :])
                        av_sb = spool.tile([64, 512], f32, tag="avsb",
                                           name="avsb")
                        nc.vector.tensor_copy(av_sb[:], av[hh][0:64, :])
                        d32s.append(d32)
                        av_sbs.append(av_sb)
                    for hh in range(2):
                        h = 2 * t + hh
                        bc_sb = spool.tile([64, 512], f32, tag="bcsb",
                                           name="bcsb")
                        nc.gpsimd.partition_broadcast(bc_sb[:], d32s[hh][:])
                        rb = spool.tile([64, 512], f32, tag="rb", name="rb")
                        nc.vector.reciprocal_approx_fast(rb[:], bc_sb[:])
                        onh = opool.tile([64, 512], f16, tag="onh", name="onh")
                        nc.vector.tensor_mul(onh[:], av_sbs[hh][:], rb[:])
                        onh_tiles[h] = onh
                        if debug:
                            nc.sync.dma_start(
                                dbg["on"][:, h * LQ + q0:h * LQ + q0 + 512],
                                onh[:])
                pending_proj.append((q0, onh_tiles))
            emit_proj()

    nc.compile()
    return nc


_NC_CACHE = {}


def _get_nc(debug=False):
    key = (debug, MM_DT)
    if key not in _NC_CACHE:
        _NC_CACHE[key] = _build_nc(debug=debug)
    return _NC_CACHE[key]


def _host_inputs(x, gamma, beta, Wq, Wk, Wv, Wp, bp):
    """Build the 8 per-core input maps."""
    x = np.asarray(x, np.float32).reshape(4, C, L)
    scale = DH ** -0.5
    mmdt = _np_mm_dt()
    shared = {
        "wqT": np.ascontiguousarray((np.asarray(Wq, np.float32) * scale).T).astype(mmdt),
        "wkT": np.ascontiguousarray(np.asarray(Wk, np.float32).T).astype(mmdt),
        "wvT": np.ascontiguousarray(np.asarray(Wv, np.float32).T).astype(mmdt),
        "wpT": np.ascontiguousarray(np.asarray(Wp, np.float32).T).astype(mmdt),
        "gamma": np.asarray(gamma, np.float32),
        "beta": np.asarray(beta, np.float32),
        "bp": np.asarray(bp, np.float32),
        "mgrp": _mgrp(),
        "mbc": _mbc(),
    }
    in_maps = []
    for c in range(N_CORES):
        n, j = c // 2, c % 2
        xf = x[n]
        xr = np.ascontiguousarray(
            np.concatenate([xf[:, j * LQ:], xf[:, :j * LQ]], axis=1))
        in_maps.append({"x": xr, **shared})
    return in_maps


def _mgrp():
    m = np.zeros((128, 16), np.float32)
    for p in range(128):
        m[p, p // 8] = 1.0 / 8.0
    return m


def _mbc():
    m = np.zeros((16, 128), np.float32)
    for p in range(128):
        m[p // 8, p] = 1.0
    return m


def _assemble(results):
    y = np.zeros((4, C, L), np.float32)
    for c in range(N_CORES):
        n, j = c // 2, c % 2
        y[n][:, j * LQ:(j + 1) * LQ] = results[c]["y"]
    return y.reshape(4, C, 64, 64)


def kernel(x, gamma, beta, Wq, Wk, Wv, Wp, bp):
    from concourse.bass_utils import run_bass_kernel_spmd

    nc = _get_nc()
    in_maps = _host_inputs(x, gamma, beta, Wq, Wk, Wv, Wp, bp)
    res = run_bass_kernel_spmd(nc, in_maps, core_ids=list(range(N_CORES)))
    return _assemble(res.results)


# revision 14
# speedup vs baseline: 2.8689x; 1.1858x over previous
"""Trainium2 Bass kernel for nn_AttentionBlock (GroupNorm + 4-head self-attention
over a [4, 256, 64, 64] image batch).

Sharding: 8 cores = (batch n in 0..3) x (query-half j in 0..1). Each core gets
batch n's full [256, 4096] feature map, spatially rolled by j*2048 so that the
core's 2048 query positions are always columns 0..2047 (GroupNorm stats and
softmax sums are invariant to a consistent spatial permutation). Each core
returns its [256, 2048] output slice; the host reassembles.

Numerics: GroupNorm in fp32; all large matmuls in fp16 (1 cycle/row on the PE,
fp32 PSUM accumulation). Softmax runs without the max-subtraction pass (scores
are ~N(0,1); |s| < 15 by a huge margin, exp stays in fp16/fp32 range), with the
denominator produced by an extra all-ones column in the V operand so the AV
matmul accumulates it for free.
"""

import os
import numpy as np

C = 256
L = 4096
LQ = 2048
HEADS = 4
DH = 64
GROUPS = 32
EPS = 1e-5
N_CORES = 8

# dtype for the big matmuls: f16 (default) or bf16
MM_DT = os.environ.get("ATTN_MM_DT", "f16")


def _np_mm_dt():
    import ml_dtypes
    return np.float16 if MM_DT == "f16" else ml_dtypes.bfloat16


def _build_nc(debug=False):
    import concourse.bacc as bacc
    import concourse.tile as tile
    from concourse import mybir

    f32 = mybir.dt.float32
    f16 = mybir.dt.float16 if MM_DT == "f16" else mybir.dt.bfloat16

    AF = mybir.ActivationFunctionType
    OP = mybir.AluOpType

    nc = bacc.Bacc("TRN2", target_bir_lowering=False, debug=False,
                   num_devices=N_CORES)

    x_d = nc.dram_tensor("x", [C, L], f32, kind="ExternalInput").ap()
    wq_d = nc.dram_tensor("wqT", [C, C], f16, kind="ExternalInput").ap()
    wk_d = nc.dram_tensor("wkT", [C, C], f16, kind="ExternalInput").ap()
    wv_d = nc.dram_tensor("wvT", [C, C], f16, kind="ExternalInput").ap()
    wp_d = nc.dram_tensor("wpT", [C, C], f16, kind="ExternalInput").ap()
    gamma_d = nc.dram_tensor("gamma", [C], f32, kind="ExternalInput").ap()
    beta_d = nc.dram_tensor("beta", [C], f32, kind="ExternalInput").ap()
    bp_d = nc.dram_tensor("bp", [C], f32, kind="ExternalInput").ap()
    mgrp_d = nc.dram_tensor("mgrp", [128, 16], f32, kind="ExternalInput").ap()
    mbc_d = nc.dram_tensor("mbc", [16, 128], f32, kind="ExternalInput").ap()
    y_d = nc.dram_tensor("y", [C, LQ], f32, kind="ExternalOutput").ap()
    dbg = {}
    if debug:
        dbg["xn"] = nc.dram_tensor("dbg_xn", [C, L], f16, kind="ExternalOutput").ap()
        dbg["q"] = nc.dram_tensor("dbg_q", [C, LQ], f16, kind="ExternalOutput").ap()
        dbg["k"] = nc.dram_tensor("dbg_k", [C, L], f16, kind="ExternalOutput").ap()
        dbg["v"] = nc.dram_tensor("dbg_v", [128, 32 * HEADS * 65], f16,
                                  kind="ExternalOutput").ap()
        dbg["on"] = nc.dram_tensor("dbg_on", [64, HEADS * LQ], f16,
                                   kind="ExternalOutput").ap()

    with tile.TileContext(nc) as tc:
        with (
            tc.tile_pool(name="consts", bufs=1) as cpool,
            tc.tile_pool(name="xp", bufs=1) as xpool,
            tc.tile_pool(name="xnp", bufs=1) as xnpool,
            tc.tile_pool(name="kp", bufs=1) as kpool,
            tc.tile_pool(name="qp", bufs=1) as qpool,
            tc.tile_pool(name="vp", bufs=1) as vpool,
            tc.tile_pool(name="small", bufs=2) as spool,
            tc.tile_pool(name="ptp", bufs=4) as ptpool,
            tc.tile_pool(name="onp", bufs=10) as opool,
            tc.tile_pool(name="yp", bufs=2) as ypool,
            tc.tile_pool(name="psA", bufs=2, space="PSUM") as psA,
            tc.tile_pool(name="psB", bufs=2, space="PSUM") as psB,
        ):
            # ---------------- input DMAs ----------------
            xt = []
            for t in range(2):
                xx = xpool.tile([128, L], f32, tag=f"x{t}", name=f"x{t}")
                nc.sync.dma_start(xx[:], x_d[t * 128:(t + 1) * 128, :])
                xt.append(xx)

            wq_sb = cpool.tile([128, 2, C], f16, tag="wq")
            nc.sync.dma_start(wq_sb[:], wq_d.rearrange("(k p) o -> p k o", k=2))
            wk_sb = cpool.tile([128, 2, C], f16, tag="wk")
            nc.sync.dma_start(wk_sb[:], wk_d.rearrange("(k p) o -> p k o", k=2))
            wv_sb = cpool.tile([128, 2, C], f16, tag="wv")
            nc.sync.dma_start(wv_sb[:], wv_d.rearrange("(k p) o -> p k o", k=2))
            wp_sb = cpool.tile([64, HEADS, C], f16, tag="wp")
            nc.sync.dma_start(wp_sb[:], wp_d.rearrange("(h p) o -> p h o", h=4))

            gcol = cpool.tile([128, 2], f32, tag="gcol")
            nc.sync.dma_start(gcol[:], gamma_d.rearrange("(t p) -> p t", t=2))
            bcol = cpool.tile([128, 2], f32, tag="bcol")
            nc.sync.dma_start(bcol[:], beta_d.rearrange("(t p) -> p t", t=2))
            bpcol = cpool.tile([128, 2], f32, tag="bpcol")
            nc.sync.dma_start(bpcol[:], bp_d.rearrange("(t p) -> p t", t=2))
            mgrp_sb = cpool.tile([128, 16], f32, tag="mgrp")
            nc.sync.dma_start(mgrp_sb[:], mgrp_d[:])
            mbc_sb = cpool.tile([16, 128], f32, tag="mbc")
            nc.sync.dma_start(mbc_sb[:], mbc_d[:])

            ones_sb = cpool.tile([1, 64], f16, tag="ones")
            nc.vector.memset(ones_sb[:], 1.0)

            # ---------------- GroupNorm (fp32) ----------------
            # Per-channel mean/E[x^2] via bn_stats, group-aggregated via a tiny
            # PE matmul with the (1/8) group-membership matrix, broadcast back
            # with its transpose, applied as a per-partition affine.
            sb4 = spool.tile([128, 4], f32, tag="sb4")
            for t in range(2):
                stats = spool.tile([128, 8, 6], f32, tag="stats")
                xv = xt[t].rearrange("p (s f) -> p s f", f=512)
                for s in range(8):
                    nc.vector.bn_stats(stats[:, s, :], xv[:, s, :])
                mv = spool.tile([128, 2], f32, tag="mv")
                nc.vector.bn_aggr(mv[:], stats[:])
                nc.vector.tensor_copy(sb4[:, 2 * t:2 * t + 1], mv[:, 0:1])
                tmp = spool.tile([128, 1], f32, tag="tmp1")
                nc.vector.tensor_mul(tmp[:], mv[:, 0:1], mv[:, 0:1])
                nc.vector.tensor_add(sb4[:, 2 * t + 1:2 * t + 2], mv[:, 1:2], tmp[:])

            gps = psA.tile([16, 4], f32, tag="sc")
            nc.tensor.matmul(gps[:], mgrp_sb[:], sb4[:], start=True, stop=True)
            gsb = spool.tile([16, 4], f32, tag="gsb")
            nc.vector.tensor_copy(gsb[:], gps[:])

            # gb2 cols: mean_t0, rstd_t0, mean_t1, rstd_t1
            gb2 = spool.tile([16, 4], f32, tag="gb2")
            gw = spool.tile([16, 8], f32, tag="gw")
            for t in range(2):
                m = gsb[:, 2 * t:2 * t + 1]
                e = gsb[:, 2 * t + 1:2 * t + 2]
                msq = gw[:, t:t + 1]
                nc.vector.tensor_mul(msq, m, m)
                w = gw[:, 2 + t:3 + t]
                nc.vector.tensor_sub(w, e, msq)                  # var
                nc.vector.tensor_scalar_add(w, w, EPS)           # var + eps
                s_ = gw[:, 4 + t:5 + t]
                nc.scalar.activation(s_, w, AF.Sqrt)             # sqrt(var+eps)
                r_ = gb2[:, 2 * t + 1:2 * t + 2]
                nc.vector.reciprocal(r_, s_)                     # ~rsqrt
                # one Newton step: r *= 1.5 - 0.5 * w * r^2
                t2 = gw[:, 6 + t:7 + t]
                nc.vector.tensor_mul(t2, r_, r_)
                nc.vector.tensor_mul(t2, t2, w)
                nc.vector.tensor_scalar(t2, t2, -0.5, 1.5, op0=OP.mult, op1=OP.add)
                nc.vector.tensor_mul(r_, r_, t2)
                nc.vector.tensor_copy(gb2[:, 2 * t:2 * t + 1], m)

            bps = psA.tile([128, 4], f32, tag="sc")
            nc.tensor.matmul(bps[:], mbc_sb[:], gb2[:], start=True, stop=True)
            pcol = spool.tile([128, 4], f32, tag="pcol")
            nc.vector.tensor_copy(pcol[:], bps[:])

            xn = []
            for t in range(2):
                nc.vector.tensor_scalar(
                    xt[t][:], xt[t][:],
                    pcol[:, 2 * t:2 * t + 1], pcol[:, 2 * t + 1:2 * t + 2],
                    op0=OP.subtract, op1=OP.mult)
                x16 = xnpool.tile([128, L], f16, tag=f"xn{t}", name=f"xn{t}")
                nc.vector.tensor_scalar(
                    x16[:], xt[t][:],
                    gcol[:, t:t + 1], bcol[:, t:t + 1],
                    op0=OP.mult, op1=OP.add)
                xn.append(x16)
            if debug:
                for t in range(2):
                    nc.sync.dma_start(dbg["xn"][t * 128:(t + 1) * 128, :], xn[t][:])

            # ---------------- Q/K projections (channel-major, fp16) ------------
            # PSUM comes from the psB slots so the 6 score banks stay free and
            # attention can start as soon as k/q chunk 0 and v land. Emission
            # order: all of k, q's first lq chunk, v, then the rest of q.
            qt = [qpool.tile([128, LQ], f16, tag=f"q{t}", name=f"q{t}")
                  for t in range(2)]
            kt = [kpool.tile([128, L], f16, tag=f"k{t}", name=f"k{t}")
                  for t in range(2)]

            def proj_chunk(dst, w_sb, t, c0):
                ps = psB.tile([128, 512], f32, tag="av", name="ps")
                for kk in range(2):
                    nc.tensor.matmul(
                        ps[:],
                        w_sb[:, kk, t * 128:(t + 1) * 128],
                        xn[kk][:, c0:c0 + 512],
                        start=(kk == 0), stop=(kk == 1))
                nc.vector.tensor_copy(dst[t][:, c0:c0 + 512], ps[:])

            for t in range(2):
                for c0 in range(0, L, 512):
                    proj_chunk(kt, wk_sb, t, c0)
            for t in range(2):
                proj_chunk(qt, wq_sb, t, 0)
            if debug:
                for t in range(2):
                    nc.sync.dma_start(dbg["q"][t * 128:(t + 1) * 128, :], qt[t][:])
                    nc.sync.dma_start(dbg["k"][t * 128:(t + 1) * 128, :], kt[t][:])

            # ---------------- V projection, directly row-major ----------------
            # v_aug[p, j, h, 0:64] = v_h[m = j*128+p, d]; col 64 = 1.0 (softmax
            # denominator row produced for free by the AV matmul).
            v_aug = vpool.tile([128, 32, HEADS, 65], f16, tag="vaug")
            nc.vector.memset(v_aug[:, :, :, 64:65], 1.0)
            for j in range(32):
                ps = psB.tile([128, 256], f32, tag="av")
                for kk in range(2):
                    nc.tensor.matmul(
                        ps[:],
                        xn[kk][:, j * 128:(j + 1) * 128],
                        wv_sb[:, kk, :],
                        start=(kk == 0), stop=(kk == 1))
                nc.vector.tensor_copy(
                    v_aug[:, j, :, 0:64],
                    ps.rearrange("p (h d) -> p h d", h=HEADS))
            for t in range(2):
                for c0 in range(512, LQ, 512):
                    proj_chunk(qt, wq_sb, t, c0)
            if debug:
                nc.sync.dma_start(
                    dbg["v"][:], v_aug.rearrange("p a b c -> p (a b c)"))

            # ---------------- attention ----------------
            # Heads run in row-tiled pairs: head 2t occupies PE rows 0-63 and
            # head 2t+1 rows 64-127 (their channel-major partition homes), so
            # the two K=64 score matmuls run CONCURRENTLY in the array.
            # Score slices fill [128, 1536] PSUM tiles (3 banks, double
            # buffered); one exp ACTIVATE covers 3 slices. The softmax
            # denominator row is broadcast with a partition-broadcast DMA (no
            # PE involvement), so the PE rolls straight into the next pair.
            pending_proj = []

            def emit_proj():
                if not pending_proj:
                    return
                q0p, tiles = pending_proj.pop()
                for o in range(2):
                    yps = psB.tile([128, 512], f32, tag="av", name="yps")
                    for h in range(HEADS):
                        nc.tensor.matmul(
                            yps[:],
                            wp_sb[:, h, o * 128:(o + 1) * 128],
                            tiles[h][:],
                            start=(h == 0), stop=(h == HEADS - 1),
                            skip_group_check=True)
                    ysb = ypool.tile([128, 512], f32, tag="ysb", name="ysb")
                    nc.vector.tensor_scalar_add(ysb[:], yps[:], bpcol[:, o:o + 1])
                    nc.sync.dma_start(
                        y_d[o * 128:(o + 1) * 128, q0p:q0p + 512], ysb[:])

            for lc in range(4):            # lq chunks of 512
                q0 = lc * 512
                onh_tiles = {}
                for t in range(2):         # head pair (2t, 2t+1)
                    if t == 1:
                        # pipeline: previous lq-chunk's output projection runs
                        # between the two pairs, reusing the freed av slots and
                        # bridging the PE gap across the pair-0 normalize.
                        emit_proj()
                    av = [psB.tile([65, 512], f32, tag="av", name=f"av{hh}")
                          for hh in range(2)]
                    group = []             # [(slice_in_tile, j, hh)]
                    sc = None
                    for i in range(64):
                        j, hh = i // 2, i % 2
                        if not group:
                            ns = min(3, 64 - i)
                            sc = psA.tile([128, ns * 512], f32,
                                          tag="sc", name="sc")
                        s = len(group)
                        nc.tensor.matmul(
                            sc[:, s * 512:(s + 1) * 512],
                            kt[t][64 * hh:64 * hh + 64, j * 128:(j + 1) * 128],
                            qt[t][64 * hh:64 * hh + 64, q0:q0 + 512],
                            start=True, stop=True)
                        group.append((s, j, hh))
                        if len(group) * 512 == sc.shape[1] or i == 63:
                            ng = len(group)
                            pt = ptpool.tile([128, ng * 512], f16, tag="pt",
                                             name="pt")
                            nc.scalar.activation(pt[:], sc[:], AF.Exp)
                            for (s, ji, hi) in group:
                                nc.tensor.matmul(
                                    av[hi][:],
                                    v_aug[:, ji, 2 * t + hi, :],
                                    pt[:, s * 512:(s + 1) * 512],
                                    start=(ji == 0), stop=(ji == 31),
                                    skip_group_check=True)
                            group = []
                    # normalize: out = av[0:64] * (1 / av[64]); denominator
                    # row -> DMA partition-broadcast -> reciprocal -> multiply
                    # (DVE + DMA only; PE proceeds with the next pair).
                    # evacuate both accumulators to SBUF promptly so the
                    # PSUM slots free up for the next pair's accumulation
                    av_sbs, d32s = [], []
                    for hh in range(2):
                        d32 = spool.tile([1, 512], f32, tag="d32", name="d32")
                        nc.vector.tensor_copy(d32[:], av[hh][64:65, :])
                        av_sb = spool.tile([64, 512], f32, tag="avsb",
                                           name="avsb")
                        nc.vector.tensor_copy(av_sb[:], av[hh][0:64, :])
                        d32s.append(d32)
                        av_sbs.append(av_sb)
                    for hh in range(2):
                        h = 2 * t + hh
                        bc_sb = spool.tile([64, 512], f32, tag="bcsb",
                                           name="bcsb")
                        nc.gpsimd.partition_broadcast(bc_sb[:], d32s[hh][:])
                        rb = spool.tile([64, 512], f32, tag="rb", name="rb")
                        nc.vector.reciprocal_approx_fast(rb[:], bc_sb[:])
                        onh = opool.tile([64, 512], f16, tag="onh", name="onh")
                        nc.vector.tensor_mul(onh[:], av_sbs[hh][:], rb[:])
                        onh_tiles[h] = onh
                        if debug:
                            nc.sync.dma_start(
                                dbg["on"][:, h * LQ + q0:h * LQ + q0 + 512],
                                onh[:])
                pending_proj.append((q0, onh_tiles))
            emit_proj()

    nc.compile()
    return nc


_NC_CACHE = {}


def _get_nc(debug=False):
    key = (debug, MM_DT)
    if key not in _NC_CACHE:
        _NC_CACHE[key] = _build_nc(debug=debug)
    return _NC_CACHE[key]


def _host_inputs(x, gamma, beta, Wq, Wk, Wv, Wp, bp):
    """Build the 8 per-core input maps."""
    x = np.asarray(x, np.float32).reshape(4, C, L)
    scale = DH ** -0.5
    mmdt = _np_mm_dt()
    shared = {
        "wqT": np.ascontiguousarray((np.asarray(Wq, np.float32) * scale).T).astype(mmdt),
        "wkT": np.ascontiguousarray(np.asarray(Wk, np.float32).T).astype(mmdt),
        "wvT": np.ascontiguousarray(np.asarray(Wv, np.float32).T).astype(mmdt),
        "wpT": np.ascontiguousarray(np.asarray(Wp, np.float32).T).astype(mmdt),
        "gamma": np.asarray(gamma, np.float32),
        "beta": np.asarray(beta, np.float32),
        "bp": np.asarray(bp, np.float32),
        "mgrp": _mgrp(),
        "mbc": _mbc(),
    }
    in_maps = []
    for c in range(N_CORES):
        n, j = c // 2, c % 2
        xf = x[n]
        xr = np.ascontiguousarray(
            np.concatenate([xf[:, j * LQ:], xf[:, :j * LQ]], axis=1))
        in_maps.append({"x": xr, **shared})
    return in_maps


def _mgrp():
    m = np.zeros((128, 16), np.float32)
    for p in range(128):
        m[p, p // 8] = 1.0 / 8.0
    return m


def _mbc():
    m = np.zeros((16, 128), np.float32)
    for p in range(128):
        m[p // 8, p] = 1.0
    return m


def _assemble(results):
    y = np.zeros((4, C, L), np.float32)
    for c in range(N_CORES):
        n, j = c // 2, c % 2
        y[n][:, j * LQ:(j + 1) * LQ] = results[c]["y"]
    return y.reshape(4, C, 64, 64)


def kernel(x, gamma, beta, Wq, Wk, Wv, Wp, bp):
    from concourse.bass_utils import run_bass_kernel_spmd

    nc = _get_nc()
    in_maps = _host_inputs(x, gamma, beta, Wq, Wk, Wv, Wp, bp)
    res = run_bass_kernel_spmd(nc, in_maps, core_ids=list(range(N_CORES)))
    return _assemble(res.results)


# revision 16
# speedup vs baseline: 2.8928x; 1.0083x over previous
"""Trainium2 Bass kernel for nn_AttentionBlock (GroupNorm + 4-head self-attention
over a [4, 256, 64, 64] image batch).

Sharding: 8 cores = (batch n in 0..3) x (query-half j in 0..1). Each core gets
batch n's full [256, 4096] feature map, spatially rolled by j*2048 so that the
core's 2048 query positions are always columns 0..2047 (GroupNorm stats and
softmax sums are invariant to a consistent spatial permutation). Each core
returns its [256, 2048] output slice; the host reassembles.

Numerics: GroupNorm in fp32; all large matmuls in fp16 (1 cycle/row on the PE,
fp32 PSUM accumulation). Softmax runs without the max-subtraction pass (scores
are ~N(0,1); |s| < 15 by a huge margin, exp stays in fp16/fp32 range), with the
denominator produced by an extra all-ones column in the V operand so the AV
matmul accumulates it for free.
"""

import os
import numpy as np

C = 256
L = 4096
LQ = 2048
HEADS = 4
DH = 64
GROUPS = 32
EPS = 1e-5
N_CORES = 8

# dtype for the big matmuls: f16 (default) or bf16
MM_DT = os.environ.get("ATTN_MM_DT", "f16")


def _np_mm_dt():
    import ml_dtypes
    return np.float16 if MM_DT == "f16" else ml_dtypes.bfloat16


def _build_nc(debug=False):
    import concourse.bacc as bacc
    import concourse.tile as tile
    from concourse import mybir

    f32 = mybir.dt.float32
    f16 = mybir.dt.float16 if MM_DT == "f16" else mybir.dt.bfloat16

    AF = mybir.ActivationFunctionType
    OP = mybir.AluOpType

    nc = bacc.Bacc("TRN2", target_bir_lowering=False, debug=False,
                   num_devices=N_CORES)

    x_d = nc.dram_tensor("x", [C, L], f32, kind="ExternalInput").ap()
    wq_d = nc.dram_tensor("wqT", [C, C], f16, kind="ExternalInput").ap()
    wk_d = nc.dram_tensor("wkT", [C, C], f16, kind="ExternalInput").ap()
    wv_d = nc.dram_tensor("wvT", [C, C], f16, kind="ExternalInput").ap()
    wp_d = nc.dram_tensor("wpT", [C, C], f16, kind="ExternalInput").ap()
    gamma_d = nc.dram_tensor("gamma", [C], f32, kind="ExternalInput").ap()
    beta_d = nc.dram_tensor("beta", [C], f32, kind="ExternalInput").ap()
    bp_d = nc.dram_tensor("bp", [C], f32, kind="ExternalInput").ap()
    mgrp_d = nc.dram_tensor("mgrp", [128, 16], f32, kind="ExternalInput").ap()
    mbc_d = nc.dram_tensor("mbc", [16, 128], f32, kind="ExternalInput").ap()
    y_d = nc.dram_tensor("y", [C, LQ], f32, kind="ExternalOutput").ap()
    dbg = {}
    if debug:
        dbg["xn"] = nc.dram_tensor("dbg_xn", [C, L], f16, kind="ExternalOutput").ap()
        dbg["q"] = nc.dram_tensor("dbg_q", [C, LQ], f16, kind="ExternalOutput").ap()
        dbg["k"] = nc.dram_tensor("dbg_k", [C, L], f16, kind="ExternalOutput").ap()
        dbg["v"] = nc.dram_tensor("dbg_v", [128, 32 * HEADS * 65], f16,
                                  kind="ExternalOutput").ap()
        dbg["on"] = nc.dram_tensor("dbg_on", [64, HEADS * LQ], f16,
                                   kind="ExternalOutput").ap()

    with tile.TileContext(nc) as tc:
        with (
            tc.tile_pool(name="consts", bufs=1) as cpool,
            tc.tile_pool(name="xp", bufs=1) as xpool,
            tc.tile_pool(name="xnp", bufs=1) as xnpool,
            tc.tile_pool(name="kp", bufs=1) as kpool,
            tc.tile_pool(name="qp", bufs=1) as qpool,
            tc.tile_pool(name="vp", bufs=1) as vpool,
            tc.tile_pool(name="small", bufs=2) as spool,
            tc.tile_pool(name="ptp", bufs=4) as ptpool,
            tc.tile_pool(name="onp", bufs=10) as opool,
            tc.tile_pool(name="yp", bufs=2) as ypool,
            tc.tile_pool(name="psA", bufs=2, space="PSUM") as psA,
            tc.tile_pool(name="psB", bufs=2, space="PSUM") as psB,
        ):
            # ---------------- input DMAs ----------------
            xt = []
            for t in range(2):
                xx = xpool.tile([128, L], f32, tag=f"x{t}", name=f"x{t}")
                nc.sync.dma_start(xx[:], x_d[t * 128:(t + 1) * 128, :])
                xt.append(xx)

            wq_sb = cpool.tile([128, 2, C], f16, tag="wq")
            nc.sync.dma_start(wq_sb[:], wq_d.rearrange("(k p) o -> p k o", k=2))
            wk_sb = cpool.tile([128, 2, C], f16, tag="wk")
            nc.sync.dma_start(wk_sb[:], wk_d.rearrange("(k p) o -> p k o", k=2))
            wv_sb = cpool.tile([128, 2, C], f16, tag="wv")
            nc.sync.dma_start(wv_sb[:], wv_d.rearrange("(k p) o -> p k o", k=2))
            wp_sb = cpool.tile([64, HEADS, C], f16, tag="wp")
            nc.sync.dma_start(wp_sb[:], wp_d.rearrange("(h p) o -> p h o", h=4))

            gcol = cpool.tile([128, 2], f32, tag="gcol")
            nc.sync.dma_start(gcol[:], gamma_d.rearrange("(t p) -> p t", t=2))
            bcol = cpool.tile([128, 2], f32, tag="bcol")
            nc.sync.dma_start(bcol[:], beta_d.rearrange("(t p) -> p t", t=2))
            bpcol = cpool.tile([128, 2], f32, tag="bpcol")
            nc.sync.dma_start(bpcol[:], bp_d.rearrange("(t p) -> p t", t=2))
            mgrp_sb = cpool.tile([128, 16], f32, tag="mgrp")
            nc.sync.dma_start(mgrp_sb[:], mgrp_d[:])
            mbc_sb = cpool.tile([16, 128], f32, tag="mbc")
            nc.sync.dma_start(mbc_sb[:], mbc_d[:])

            ones_sb = cpool.tile([1, 64], f16, tag="ones")
            nc.vector.memset(ones_sb[:], 1.0)

            # ---------------- GroupNorm (fp32) ----------------
            # Per-channel mean/E[x^2] via bn_stats, group-aggregated via a tiny
            # PE matmul with the (1/8) group-membership matrix, broadcast back
            # with its transpose, applied as a per-partition affine.
            sb4 = spool.tile([128, 4], f32, tag="sb4")
            for t in range(2):
                stats = spool.tile([128, 8, 6], f32, tag="stats")
                xv = xt[t].rearrange("p (s f) -> p s f", f=512)
                for s in range(8):
                    nc.vector.bn_stats(stats[:, s, :], xv[:, s, :])
                mv = spool.tile([128, 2], f32, tag="mv")
                nc.vector.bn_aggr(mv[:], stats[:])
                nc.vector.tensor_copy(sb4[:, 2 * t:2 * t + 1], mv[:, 0:1])
                tmp = spool.tile([128, 1], f32, tag="tmp1")
                nc.vector.tensor_mul(tmp[:], mv[:, 0:1], mv[:, 0:1])
                nc.vector.tensor_add(sb4[:, 2 * t + 1:2 * t + 2], mv[:, 1:2], tmp[:])

            gps = psA.tile([16, 4], f32, tag="sc")
            nc.tensor.matmul(gps[:], mgrp_sb[:], sb4[:], start=True, stop=True)
            gsb = spool.tile([16, 4], f32, tag="gsb")
            nc.vector.tensor_copy(gsb[:], gps[:])

            # gb2 cols: mean_t0, rstd_t0, mean_t1, rstd_t1
            gb2 = spool.tile([16, 4], f32, tag="gb2")
            gw = spool.tile([16, 8], f32, tag="gw")
            for t in range(2):
                m = gsb[:, 2 * t:2 * t + 1]
                e = gsb[:, 2 * t + 1:2 * t + 2]
                msq = gw[:, t:t + 1]
                nc.vector.tensor_mul(msq, m, m)
                w = gw[:, 2 + t:3 + t]
                nc.vector.tensor_sub(w, e, msq)                  # var
                nc.vector.tensor_scalar_add(w, w, EPS)           # var + eps
                s_ = gw[:, 4 + t:5 + t]
                nc.scalar.activation(s_, w, AF.Sqrt)             # sqrt(var+eps)
                r_ = gb2[:, 2 * t + 1:2 * t + 2]
                nc.vector.reciprocal(r_, s_)                     # ~rsqrt
                # one Newton step: r *= 1.5 - 0.5 * w * r^2
                t2 = gw[:, 6 + t:7 + t]
                nc.vector.tensor_mul(t2, r_, r_)
                nc.vector.tensor_mul(t2, t2, w)
                nc.vector.tensor_scalar(t2, t2, -0.5, 1.5, op0=OP.mult, op1=OP.add)
                nc.vector.tensor_mul(r_, r_, t2)
                nc.vector.tensor_copy(gb2[:, 2 * t:2 * t + 1], m)

            bps = psA.tile([128, 4], f32, tag="sc")
            nc.tensor.matmul(bps[:], mbc_sb[:], gb2[:], start=True, stop=True)
            pcol = spool.tile([128, 4], f32, tag="pcol")
            nc.vector.tensor_copy(pcol[:], bps[:])

            # fused per-channel affine: xn = x*A + B with A = rstd*gamma,
            # B = beta - mean*rstd*gamma; tile 0 on DVE, tile 1 on ScalarE
            # so the two applies run in parallel.
            ab = spool.tile([128, 4], f32, tag="ab")   # A_t0, B_t0, A_t1, B_t1
            for t in range(2):
                a_ = ab[:, 2 * t:2 * t + 1]
                b_ = ab[:, 2 * t + 1:2 * t + 2]
                nc.vector.tensor_mul(a_, pcol[:, 2 * t + 1:2 * t + 2],
                                     gcol[:, t:t + 1])
                nc.vector.tensor_mul(b_, pcol[:, 2 * t:2 * t + 1], a_)
                nc.vector.tensor_sub(b_, bcol[:, t:t + 1], b_)
            xn = []
            for t in range(2):
                x16 = xnpool.tile([128, L], f16, tag=f"xn{t}", name=f"xn{t}")
                if t == 0:
                    nc.vector.tensor_scalar(
                        x16[:], xt[t][:],
                        ab[:, 0:1], ab[:, 1:2], op0=OP.mult, op1=OP.add)
                else:
                    nc.scalar.activation(
                        x16[:], xt[t][:], AF.Identity,
                        bias=ab[:, 3:4], scale=ab[:, 2:3])
                xn.append(x16)
            if debug:
                for t in range(2):
                    nc.sync.dma_start(dbg["xn"][t * 128:(t + 1) * 128, :], xn[t][:])

            # ---------------- Q/K projections (channel-major, fp16) ------------
            # PSUM comes from the psB slots so the 6 score banks stay free and
            # attention can start as soon as k/q chunk 0 and v land. Emission
            # order: all of k, q's first lq chunk, v, then the rest of q.
            qt = [qpool.tile([128, LQ], f16, tag=f"q{t}", name=f"q{t}")
                  for t in range(2)]
            kt = [kpool.tile([128, L], f16, tag=f"k{t}", name=f"k{t}")
                  for t in range(2)]

            def proj_chunk(dst, w_sb, t, c0):
                ps = psB.tile([128, 512], f32, tag="av", name="ps")
                for kk in range(2):
                    nc.tensor.matmul(
                        ps[:],
                        w_sb[:, kk, t * 128:(t + 1) * 128],
                        xn[kk][:, c0:c0 + 512],
                        start=(kk == 0), stop=(kk == 1))
                nc.vector.tensor_copy(dst[t][:, c0:c0 + 512], ps[:])

            for t in range(2):
                for c0 in range(0, L, 512):
                    proj_chunk(kt, wk_sb, t, c0)
            for t in range(2):
                proj_chunk(qt, wq_sb, t, 0)

            # ---------------- V projection, directly row-major ----------------
            # v_aug[p, j, h, 0:64] = v_h[m = j*128+p, d]; col 64 = 1.0 (softmax
            # denominator row produced for free by the AV matmul).
            v_aug = vpool.tile([128, 32, HEADS, 65], f16, tag="vaug")
            nc.vector.memset(v_aug[:, :, :, 64:65], 1.0)
            for j in range(32):
                ps = psB.tile([128, 256], f32, tag="av")
                for kk in range(2):
                    nc.tensor.matmul(
                        ps[:],
                        xn[kk][:, j * 128:(j + 1) * 128],
                        wv_sb[:, kk, :],
                        start=(kk == 0), stop=(kk == 1))
                nc.vector.tensor_copy(
                    v_aug[:, j, :, 0:64],
                    ps.rearrange("p (h d) -> p h d", h=HEADS))
            for t in range(2):
                for c0 in range(512, LQ, 512):
                    proj_chunk(qt, wq_sb, t, c0)
            if debug:
                for t in range(2):
                    nc.sync.dma_start(dbg["q"][t * 128:(t + 1) * 128, :], qt[t][:])
                    nc.sync.dma_start(dbg["k"][t * 128:(t + 1) * 128, :], kt[t][:])
            if debug:
                nc.sync.dma_start(
                    dbg["v"][:], v_aug.rearrange("p a b c -> p (a b c)"))

            # ---------------- attention ----------------
            # Heads run in row-tiled pairs: head 2t occupies PE rows 0-63 and
            # head 2t+1 rows 64-127 (their channel-major partition homes), so
            # the two K=64 score matmuls run CONCURRENTLY in the array.
            # Score slices fill [128, 1536] PSUM tiles (3 banks, double
            # buffered); one exp ACTIVATE covers 3 slices. The softmax
            # denominator row is broadcast with a partition-broadcast DMA (no
            # PE involvement), so the PE rolls straight into the next pair.
            pending_proj = []

            def emit_proj():
                if not pending_proj:
                    return
                q0p, tiles = pending_proj.pop()
                for o in range(2):
                    yps = psB.tile([128, 512], f32, tag="av", name="yps")
                    for h in range(HEADS):
                        nc.tensor.matmul(
                            yps[:],
                            wp_sb[:, h, o * 128:(o + 1) * 128],
                            tiles[h][:],
                            start=(h == 0), stop=(h == HEADS - 1),
                            skip_group_check=True)
                    ysb = ypool.tile([128, 512], f32, tag="ysb", name="ysb")
                    nc.vector.tensor_scalar_add(ysb[:], yps[:], bpcol[:, o:o + 1])
                    nc.sync.dma_start(
                        y_d[o * 128:(o + 1) * 128, q0p:q0p + 512], ysb[:])

            for lc in range(4):            # lq chunks of 512
                q0 = lc * 512
                onh_tiles = {}
                for t in range(2):         # head pair (2t, 2t+1)
                    if t == 1:
                        # pipeline: previous lq-chunk's output projection runs
                        # between the two pairs, reusing the freed av slots and
                        # bridging the PE gap across the pair-0 normalize.
                        emit_proj()
                    av = [psB.tile([65, 512], f32, tag="av", name=f"av{hh}")
                          for hh in range(2)]
                    group = []             # [(slice_in_tile, j, hh)]
                    sc = None
                    for i in range(64):
                        j, hh = i // 2, i % 2
                        if not group:
                            ns = min(3, 64 - i)
                            sc = psA.tile([128, ns * 512], f32,
                                          tag="sc", name="sc")
                        s = len(group)
                        nc.tensor.matmul(
                            sc[:, s * 512:(s + 1) * 512],
                            kt[t][64 * hh:64 * hh + 64, j * 128:(j + 1) * 128],
                            qt[t][64 * hh:64 * hh + 64, q0:q0 + 512],
                            start=True, stop=True)
                        group.append((s, j, hh))
                        if len(group) * 512 == sc.shape[1] or i == 63:
                            ng = len(group)
                            pt = ptpool.tile([128, ng * 512], f16, tag="pt",
                                             name="pt")
                            nc.scalar.activation(pt[:], sc[:], AF.Exp)
                            for (s, ji, hi) in group:
                                nc.tensor.matmul(
                                    av[hi][:],
                                    v_aug[:, ji, 2 * t + hi, :],
                                    pt[:, s * 512:(s + 1) * 512],
                                    start=(ji == 0), stop=(ji == 31),
                                    skip_group_check=True)
                            group = []
                    # normalize: out = av[0:64] * (1 / av[64]); denominator
                    # row -> DMA partition-broadcast -> reciprocal -> multiply
                    # (DVE + DMA only; PE proceeds with the next pair).
                    # evacuate both accumulators to SBUF promptly so the
                    # PSUM slots free up for the next pair's accumulation
                    av_sbs, d32s = [], []
                    for hh in range(2):
                        d32 = spool.tile([1, 512], f32, tag="d32", name="d32")
                        nc.vector.tensor_copy(d32[:], av[hh][64:65, :])
                        av_sb = spool.tile([64, 512], f32, tag="avsb",
                                           name="avsb")
                        nc.vector.tensor_copy(av_sb[:], av[hh][0:64, :])
                        d32s.append(d32)
                        av_sbs.append(av_sb)
                    for hh in range(2):
                        h = 2 * t + hh
                        bc_sb = spool.tile([64, 512], f32, tag="bcsb",
                                           name="bcsb")
                        nc.gpsimd.partition_broadcast(bc_sb[:], d32s[hh][:])
                        rb = spool.tile([64, 512], f32, tag="rb", name="rb")
                        nc.vector.reciprocal_approx_fast(rb[:], bc_sb[:])
                        onh = opool.tile([64, 512], f16, tag="onh", name="onh")
                        nc.vector.tensor_mul(onh[:], av_sbs[hh][:], rb[:])
                        onh_tiles[h] = onh
                        if debug:
                            nc.sync.dma_start(
                                dbg["on"][:, h * LQ + q0:h * LQ + q0 + 512],
                                onh[:])
                pending_proj.append((q0, onh_tiles))
            emit_proj()

    nc.compile()
    return nc


_NC_CACHE = {}


def _get_nc(debug=False):
    key = (debug, MM_DT)
    if key not in _NC_CACHE:
        _NC_CACHE[key] = _build_nc(debug=debug)
    return _NC_CACHE[key]


def _host_inputs(x, gamma, beta, Wq, Wk, Wv, Wp, bp):
    """Build the 8 per-core input maps."""
    x = np.asarray(x, np.float32).reshape(4, C, L)
    scale = DH ** -0.5
    mmdt = _np_mm_dt()
    shared = {
        "wqT": np.ascontiguousarray((np.asarray(Wq, np.float32) * scale).T).astype(mmdt),
        "wkT": np.ascontiguousarray(np.asarray(Wk, np.float32).T).astype(mmdt),
        "wvT": np.ascontiguousarray(np.asarray(Wv, np.float32).T).astype(mmdt),
        "wpT": np.ascontiguousarray(np.asarray(Wp, np.float32).T).astype(mmdt),
        "gamma": np.asarray(gamma, np.float32),
        "beta": np.asarray(beta, np.float32),
        "bp": np.asarray(bp, np.float32),
        "mgrp": _mgrp(),
        "mbc": _mbc(),
    }
    in_maps = []
    for c in range(N_CORES):
        n, j = c // 2, c % 2
        xf = x[n]
        xr = np.ascontiguousarray(
            np.concatenate([xf[:, j * LQ:], xf[:, :j * LQ]], axis=1))
        in_maps.append({"x": xr, **shared})
    return in_maps


def _mgrp():
    m = np.zeros((128, 16), np.float32)
    for p in range(128):
        m[p, p // 8] = 1.0 / 8.0
    return m


def _mbc():
    m = np.zeros((16, 128), np.float32)
    for p in range(128):
        m[p // 8, p] = 1.0
    return m


def _assemble(results):
    y = np.zeros((4, C, L), np.float32)
    for c in range(N_CORES):
        n, j = c // 2, c % 2
        y[n][:, j * LQ:(j + 1) * LQ] = results[c]["y"]
    return y.reshape(4, C, 64, 64)


def kernel(x, gamma, beta, Wq, Wk, Wv, Wp, bp):
    from concourse.bass_utils import run_bass_kernel_spmd

    nc = _get_nc()
    in_maps = _host_inputs(x, gamma, beta, Wq, Wk, Wv, Wp, bp)
    res = run_bass_kernel_spmd(nc, in_maps, core_ids=list(range(N_CORES)))
    return _assemble(res.results)
